# revision 33
# baseline (speedup 1.0000x reference)
"""CGCNN message-passing kernel for 8 Trainium2 NeuronCores.

Strategy (v3 — merged one-hot+edge-feat matmul, 108-node dst blocks):
  - Nodes partitioned contiguously across 8 cores (NPC nodes each, padded).
  - Edges sharded by dst owner, grouped by 108-node dst block, laid out
    BUCKET-MAJOR: for src-bucket k (32768-node windows, int16 gather range),
    for dst-block b, quota[b,k] slots (multiple of 128, max over cores so a
    single SPMD program fits all cores). Groups of 128 slots are
    (bucket, block)-pure.
  - hs gather: per (bucket, chunk of CHUNK_G groups) dma_gather on 2
    alternating SWDGE queues (~5.5 ns/idx measured; desc-gen is the cost).
  - z per group in ONE matmul: lhsT = DE_g = [one-hot dst (108 rows);
    ef^T (20 rows)], rhs = hdwe_b = [hd block (108 rows); we (20 rows)]:
    z = hd[dst] + ef@we in a single K=128 pass. 108 + ED(=20) = 128 exactly.
  - Gathered hs injected by DVE (z_sbuf = z_psum + hsg), not a PE matmul.
  - msg = sigmoid(zg)*softplus(zf) with the gate sign folded into weights
    (zg' = -zg): a = exp(z'), bd = ln(1+a), e = exp(-b_gate), msg = e*d.
  - Aggregation: agg^T[feat, nodes] += msg_j^T @ S_j (N=108) in PSUM per
    (bucket, block) run; runs flushed (copy/add) into an SBUF-resident
    agg accumulator; BN stats reduced from SBUF at each block's last run.
  - Fused U+H pass per 128-node tile: hnew = relu(h + bn(agg)) then
    hs/hd = hnew @ [wsrc|wdst] in one pass (embed fused the same way for
    layer 0). hd goes to DRAM and is reloaded 108-block-tiled next layer.
  - A post-pass rewrites compiler-inserted ACT_TABLE_LOADs to the shared
    natural_log_exp table set and dedups them.
  - Mean-pool via one-hot matmul + AllReduce; FC head replicated.

kernel(**inputs) -> np.ndarray (B, 1) float32. Self-contained (shapes
hardcoded below; no file reads).
"""
import sys

sys.path.insert(0, "/opt/trn_rl_repo")

import numpy as np
from ml_dtypes import bfloat16

import concourse.bacc as bacc
import concourse.bass as bass
import concourse.mybir as mybir
import concourse.tile as tile
from concourse.bass_utils import run_bass_kernel_spmd
from concourse.masks import make_identity

F32 = mybir.dt.float32
BF16 = mybir.dt.bfloat16
I32 = mybir.dt.int32
AF = mybir.ActivationFunctionType
ALU = mybir.AluOpType

NCORES = 8
GROUP = 128
SUPER = 6            # groups per supertile: z-PSUM = 3 banks, double buffered
CHUNK_G = 24         # groups per dma_gather call (must be multiple of SUPER)
BS = 108             # dst-block size; BS + ED = 128 (merged matmul rows)
EPS = 1e-5

_NL_EXP_SET_ID = 6


def _patch_act_table_loads():
    if getattr(bacc.Bacc, "_atl_patched", False):
        return
    orig = bacc.Bacc.insert_act_table_loads

    def patched(self):
        orig(self)
        for blk in self.main_func.blocks:
            first_seen = False
            drop = []
            for ins in blk.instructions:
                if not isinstance(ins, mybir.InstLoadActFuncSet):
                    continue
                ins.act_func_set_id = _NL_EXP_SET_ID
                si = getattr(ins, "sync_info", None)
                has_sync = si is not None and (
                    len(getattr(si, "on_wait", [])) > 0
                    or len(getattr(si, "on_update", [])) > 0)
                if not first_seen or has_sync:
                    first_seen = True
                    continue
                drop.append(ins)
            if drop:
                dropset = set(map(id, drop))
                blk.instructions[:] = [
                    i for i in blk.instructions if id(i) not in dropset]

    bacc.Bacc.insert_act_table_loads = patched
    bacc.Bacc._atl_patched = True


_patch_act_table_loads()


class Cfg:
    def __init__(self, N, E, B, ND, ED, H, NL):
        self.N, self.E, self.B = N, E, B
        self.ND, self.ED, self.H, self.NL = ND, ED, H, NL
        per = (N + NCORES - 1) // NCORES
        self.NPC = (per + 127) // 128 * 128
        self.NB = self.NPC // 128          # 128-node tiles (U/H passes)
        self.NB8 = -(-self.NPC // BS)      # 108-node dst blocks (edge phase)
        self.NPAD = self.NPC * NCORES


DEFAULT_CFG = Cfg(N=100000, E=1600000, B=128, ND=91, ED=20, H=128, NL=3)


# --------------------------------------------------------------------------
# Host-side preprocessing
# --------------------------------------------------------------------------
def _preprocess(cfg, inputs):
    N, B, H, ED, ND, NL = cfg.N, cfg.B, cfg.H, cfg.ED, cfg.ND, cfg.NL
    NPC, NB8 = cfg.NPC, cfg.NB8

    src = np.asarray(inputs["edge_index"][0], np.int64)
    dst = np.asarray(inputs["edge_index"][1], np.int64)
    ef32 = np.asarray(inputs["edge_feats"], np.float32)
    batch_np = np.asarray(inputs["batch"], np.int64)

    order = np.argsort(dst, kind="stable")
    src_s, dst_s, e_s = src[order], dst[order], order

    # hs tables are AllGathered in TWO halves (overlap with the update
    # pass): node n -> (core c, local r); half = r >= NPC//2; table row =
    # c*(NPC//2) + r%(NPC//2). Each half-table (NPAD//2 rows) splits into
    # 2 gather buckets of BK rows (int16 range).
    NBK = 4
    HALF = NPC // 2
    BK = NCORES * HALF // 2          # 25088 rows/bucket

    def table_row_bucket(n):
        c = n // NPC
        r = n - c * NPC
        half = (r >= HALF).astype(np.int64)
        row = c * HALF + (r - half * HALF)
        k = half * 2 + row // BK
        rel = row - (row // BK) * BK
        return k, rel

    starts = np.searchsorted(dst_s, np.arange(NCORES) * NPC)
    ends = np.searchsorted(dst_s, np.arange(NCORES) * NPC + NPC)

    src_k, src_rel = table_row_bucket(src_s)

    # per-core, per-block, per-bucket edge counts
    cnt = np.zeros((NCORES, NB8, NBK), np.int64)
    core_block = []
    for c in range(NCORES):
        lo, hi = int(starts[c]), int(ends[c])
        cdstl = dst_s[lo:hi] - c * NPC
        cblk = cdstl // BS
        bstart = np.searchsorted(cblk, np.arange(NB8))
        bend = np.searchsorted(cblk, np.arange(NB8) + 1)
        core_block.append((lo, bstart, bend))
        for b in range(NB8):
            bk = src_k[lo + bstart[b]:lo + bend[b]]
            cnt[c, b] = np.bincount(bk, minlength=NBK)

    # shared quotas: per (block, bucket), max over cores, rounded to x128
    quota = -(-cnt.max(axis=0) // GROUP) * GROUP          # [NB8, NBK]
    for b in range(NB8):
        if quota[b].sum() == 0:
            quota[b, 0] = GROUP
    G = int(quota.sum()) // GROUP

    # bucket-major group order: for k, for b, quota[b,k]//128 groups.
    grp_kb = []          # group -> (k, b)
    runs = []            # (k, b, g0, ng)
    for k in range(NBK):
        for b in range(NB8):
            ng = int(quota[b, k]) // GROUP
            if ng == 0:
                continue
            runs.append((k, b, len(grp_kb), ng))
            grp_kb += [(k, b)] * ng
    assert len(grp_kb) == G
    run_of_group_start = {g0: (k, b, ng) for (k, b, g0, ng) in runs}
    firstg, lastg = {}, {}
    for (k, b, g0, ng) in runs:
        if b not in firstg:
            firstg[b] = g0
        lastg[b] = g0
    first_run_g0 = set(firstg.values())
    last_run_g0 = set(lastg.values())

    # chunks: list of (k, g0, ng) with ng <= CHUNK_G, bucket-pure
    chunks = []
    gk0 = 0
    for k in range(NBK):
        ngk = sum(int(quota[b, k]) // GROUP for b in range(NB8))
        g = 0
        while g < ngk:
            cgn = min(CHUNK_G, ngk - g)
            chunks.append((k, gk0 + g, cgn))
            g += cgn
        gk0 += ngk

    conv_b = np.asarray(inputs["conv_b"], np.float32).copy()
    counts = np.maximum(np.bincount(batch_np, minlength=B), 1.0)
    counts_inv = (1.0 / counts).astype(np.float32).reshape(B, 1)
    wsd = [np.concatenate([np.asarray(inputs["conv_wsrc"][i], np.float32),
                           np.asarray(inputs["conv_wdst"][i], np.float32)],
                          axis=1) for i in range(NL)]             # [H, 4H]
    we_l = [np.asarray(inputs["conv_we"][i], np.float32).copy()
            for i in range(NL)]
    # fold the gate sign into the weights: zg' = -zg
    for i in range(NL):
        wsd[i][:, 0:H] *= -1.0          # wsrc gate cols
        wsd[i][:, 2 * H:3 * H] *= -1.0  # wdst gate cols
        we_l[i][:, 0:H] *= -1.0
        conv_b[i, 0:H] *= -1.0
    nf = np.asarray(inputs["node_feats"], np.float32)

    def wrap16(flat):
        n = flat.shape[0]
        w = flat.reshape(n // 16, 16).T
        return np.tile(w, (8, 1)).astype(np.int16)

    hs_cols = int(quota.sum()) // 16                      # total idx cols
    in_maps = []
    for c in range(NCORES):
        lo, bstart, bend = core_block[c]
        # interleaved per group: cols [g*256, g*256+128) = DE_g,
        # [g*256+128, (g+1)*256) = S_g — one DMA per supertile
        DES = np.zeros((128, G * 256), np.float32)
        slot_rel = np.zeros(G * 128, np.int64)     # in-bucket table offset
        for (k, b, g0, ngr) in runs:
            eb = slice(lo + bstart[b], lo + bend[b])
            eb_k = src_k[eb]
            eb_rel = src_rel[eb]
            eb_dstl = dst_s[eb] - c * NPC
            eb_e = e_s[eb]
            sel = eb_k == k
            erel, ed, ee = eb_rel[sel], eb_dstl[sel], eb_e[sel]
            q = ngr * 128
            n_e = erel.shape[0]
            assert n_e <= q, (c, b, k, n_e, q)
            off = g0 * 128
            slot_rel[off:off + n_e] = erel
            slot_rel[off + n_e:off + q] = 0            # pad: bucket base row
            ii = np.arange(n_e)
            p = ii % 128
            gg = g0 + ii // 128
            s_in_b = ed - b * BS
            cols = gg * 128 + p
            # one-hot scatter: S[p, s] at g*256+128+s, DE[s, p] at g*256+p
            DES[p, gg * 256 + 128 + s_in_b] = 1.0
            DES[s_in_b, gg * 256 + p] = 1.0
            DES[BS:BS + ED, :][:, gg * 256 + p] = ef32[ee].T

        hs_idx = np.zeros((128, hs_cols), np.int16)
        hcol = 0
        for (k, g0, cgn) in chunks:
            q = cgn * 128
            rel = slot_rel[g0 * 128:g0 * 128 + q].astype(np.int16)
            hs_idx[:, hcol:hcol + q // 16] = wrap16(rel)
            hcol += q // 16
        assert hcol == hs_cols

        NB = cfg.NB
        Bt = np.zeros((128, NB * B), np.float32)
        for t in range(NB):
            ids = np.arange(c * NPC + t * 128, c * NPC + t * 128 + 128)
            valid = ids < N
            gv = np.where(valid, batch_np[np.minimum(ids, N - 1)], 0)
            Bt[np.arange(128)[valid], t * B + gv[valid]] = 1.0

        nfT = np.zeros((ND, NPC), np.float32)  # cast to bf16 below
        n_real = min(NPC, max(0, N - c * NPC))
        if n_real > 0:
            nfT[:, :n_real] = nf[c * NPC:c * NPC + n_real].T

        m = {
            "nfT": nfT.astype(bfloat16),
            "hs_idx": hs_idx,
            "DES": DES.astype(bfloat16),
            "Btiles": Bt.astype(bfloat16),
            "embed_w": np.asarray(inputs["embed_w"], np.float32),
            "embed_b": np.asarray(inputs["embed_b"], np.float32).reshape(H, 1),
            "counts_inv": counts_inv,
            "fc1_w": np.asarray(inputs["fc1_w"], np.float32),
            "fc1_b": np.asarray(inputs["fc1_b"], np.float32).reshape(H, 1),
            "fc_g": np.asarray(inputs["fc_bn_gamma"], np.float32).reshape(H, 1),
            "fc_be": np.asarray(inputs["fc_bn_beta"], np.float32).reshape(H, 1),
            "out_w": np.asarray(inputs["out_w"], np.float32).reshape(H, 1),
        }
        for i in range(NL):
            m[f"wsd{i}"] = wsd[i]
            # we tiled per dst block for the merged rhs (rows BS..BS+ED)
            m[f"weB{i}"] = np.tile(we_l[i].astype(np.float32),
                                   (1, NB8)).astype(bfloat16)  # [ED, NB8*2H]
            m[f"biasb{i}"] = np.tile(conv_b[i][None, :], (128, 1))
            m[f"bn_g{i}"] = np.asarray(inputs["bn_gamma"][i], np.float32).reshape(H, 1)
            m[f"bn_b{i}"] = np.asarray(inputs["bn_beta"][i], np.float32).reshape(H, 1)
        in_maps.append(m)

    meta = dict(G=G, BK=BK, NBK=NBK,
                grp_kb=grp_kb, chunks=chunks,
                run_start=run_of_group_start,
                first_run_g0=first_run_g0, last_run_g0=last_run_g0,
                out_b=float(np.asarray(inputs["out_b"]).reshape(-1)[0]))
    return meta, in_maps


# --------------------------------------------------------------------------
# Device program
# --------------------------------------------------------------------------
def _build(cfg, meta, debug=False):
    N, B, H, ED, ND, NL = cfg.N, cfg.B, cfg.H, cfg.ED, cfg.ND, cfg.NL
    NPC, NB, NB8, NPAD = cfg.NPC, cfg.NB, cfg.NB8, cfg.NPAD
    G = meta["G"]
    BK, NBK = meta["BK"], meta["NBK"]
    grp_kb = meta["grp_kb"]
    chunks = meta["chunks"]
    run_start = meta["run_start"]
    first_run_g0 = meta["first_run_g0"]
    last_run_g0 = meta["last_run_g0"]
    hs_cols = G * 128 // 16
    RG = [list(range(NCORES))]

    nc = bacc.Bacc("TRN2", target_bir_lowering=False, debug=False,
                   num_devices=NCORES, num_swdge_queues=2)

    def inp(name, shape, dt=F32):
        return nc.dram_tensor(name, shape, dt, kind="ExternalInput")

    nfT_d = inp("nfT", [ND, NPC], BF16)
    hsix_d = inp("hs_idx", [128, hs_cols], mybir.dt.int16)
    DES_d = inp("DES", [128, G * 256], BF16)
    Bt_d = inp("Btiles", [128, NB * B], BF16)
    ew_d = inp("embed_w", [ND, H])
    eb_d = inp("embed_b", [H, 1])
    cinv_d = inp("counts_inv", [B, 1])
    fc1w_d = inp("fc1_w", [H, H])
    fc1b_d = inp("fc1_b", [H, 1])
    fcg_d = inp("fc_g", [H, 1])
    fcb_d = inp("fc_be", [H, 1])
    outw_d = inp("out_w", [H, 1])
    wsd_d = [inp(f"wsd{i}", [H, 4 * H]) for i in range(NL)]
    weB_d = [inp(f"weB{i}", [ED, NB8 * 2 * H], BF16) for i in range(NL)]
    bb_d = [inp(f"biasb{i}", [128, 2 * H]) for i in range(NL)]
    bng_d = [inp(f"bn_g{i}", [H, 1]) for i in range(NL)]
    bnb_d = [inp(f"bn_b{i}", [H, 1]) for i in range(NL)]
    out_d = nc.dram_tensor("out", [B, 1], F32, kind="ExternalOutput")

    with tile.TileContext(nc) as tc:
        with (
            tc.tile_pool(name="const", bufs=1) as cp,
            tc.tile_pool(name="dram", bufs=1, space="DRAM") as dr,
        ):
            HALF = NPC // 2
            HTILES = HALF // 128     # 128-node tiles per half
            hT_dram = dr.tile([128, NPC], BF16, tag="hT")
            # 108-tiled hd table; rows NPC..NB8*BS are padding, zeroed once
            # (one-hot matmul multiplies them by 0 — must be finite)
            hd_dram = dr.tile([NB8 * BS, 2 * H], BF16, tag="hdd")
            hsag_in_l, hs_full_l, stat_in_l, stat_out_l = [], [], [], []
            for i in range(NL):
                ha = dr.tile([HALF, 2 * H], BF16, tag=f"hsina{i}",
                             name=f"hsina{i}")
                hb = dr.tile([HALF, 2 * H], BF16, tag=f"hsinb{i}",
                             name=f"hsinb{i}")
                hsag_in_l.append((ha, hb))
                fa = dr.tile([NPAD // 2, 2 * H], BF16, tag=f"hsfa{i}",
                             addr_space="Shared", name=f"hsfa{i}")
                fb = dr.tile([NPAD // 2, 2 * H], BF16, tag=f"hsfb{i}",
                             addr_space="Shared", name=f"hsfb{i}")
                hs_full_l.append((fa, fb))
                sti_t = dr.tile([H, 2], F32, tag=f"stin{i}", name=f"stin{i}")
                stat_in_l.append(sti_t)
                sto_t = dr.tile([H, 2], F32, tag=f"stout{i}", name=f"stout{i}")
                stat_out_l.append(sto_t)
            g_in = dr.tile([B, H], F32, tag="gin")
            g_out = dr.tile([B, H], F32, tag="gout")

            # ---- resident constants ----
            ew_sb = cp.tile([128, H], BF16, tag="ew")
            nc.gpsimd.dma_start(ew_sb[:ND, :], ew_d[:, :])
            eb_sb = cp.tile([H, 1], F32, tag="eb")
            nc.sync.dma_start(eb_sb[:, :], eb_d[:, :])
            wsd_sb, bb_sb, bng_sb, bnb_sb = [], [], [], []
            for i in range(NL):
                w = cp.tile([H, 4 * H], BF16, tag=f"wsd{i}")
                nc.gpsimd.dma_start(w[:, :], wsd_d[i][:, :])
                wsd_sb.append(w)
                w = cp.tile([128, 2 * H], F32, tag=f"bb{i}")
                nc.sync.dma_start(w[:, :], bb_d[i][:, :])
                bb_sb.append(w)
                w = cp.tile([H, 1], F32, tag=f"bng{i}")
                nc.sync.dma_start(w[:, :], bng_d[i][:, :])
                bng_sb.append(w)
                w = cp.tile([H, 1], F32, tag=f"bnb{i}")
                nc.sync.dma_start(w[:, :], bnb_d[i][:, :])
                bnb_sb.append(w)
            cinv_sb = cp.tile([B, 1], F32, tag="cinv")
            nc.sync.dma_start(cinv_sb[:, :], cinv_d[:, :])
            fc1w_sb = cp.tile([H, H], F32, tag="fc1w")
            nc.sync.dma_start(fc1w_sb[:, :], fc1w_d[:, :])
            fc1b_sb = cp.tile([H, 1], F32, tag="fc1b")
            nc.sync.dma_start(fc1b_sb[:, :], fc1b_d[:, :])
            fcg_sb = cp.tile([H, 1], F32, tag="fcg")
            nc.sync.dma_start(fcg_sb[:, :], fcg_d[:, :])
            fcb_sb = cp.tile([H, 1], F32, tag="fcb")
            nc.sync.dma_start(fcb_sb[:, :], fcb_d[:, :])
            outw_sb = cp.tile([H, 1], F32, tag="outw")
            nc.sync.dma_start(outw_sb[:, :], outw_d[:, :])
            id_bf = cp.tile([128, 128], BF16, tag="idbf")
            make_identity(nc, id_bf[:])
            id_f32 = cp.tile([128, 128], F32, tag="idf32")
            make_identity(nc, id_f32[:])
            scsh_sb = cp.tile([H, 2], F32, tag="scsh")
            # merged rhs: rows 0..BS-1 hd per block, rows BS..127 we
            hdwe_sb = cp.tile([128, NB8 * 2 * H], BF16, tag="hdwe")
            # SBUF-resident agg accumulator [feat, padded local nodes]
            agg_sb = cp.tile([128, NB8 * BS], F32, tag="aggsb")
            zpad = cp.tile([128, 2 * H], BF16, tag="zpad")
            nc.vector.memset(zpad[:], 0.0)
            nc.sync.dma_start(hd_dram[NPC:NB8 * BS, :],
                              zpad[:NB8 * BS - NPC, :])

            def hs_hd_from_psum(pool, ps, t, li):
                """ps [128, 4H] = h@[wsrc|wdst]; emit hs row + hd to DRAM.
                Stores ride the gpsimd DMA queue (sync is the bottleneck in
                the fused update pass)."""
                hs_row = pool.tile([128, 2 * H], BF16, tag="hsr")
                nc.scalar.activation(hs_row[:], ps[:, :2 * H], AF.Copy)
                hd_row = pool.tile([128, 2 * H], BF16, tag="hdr")
                nc.vector.tensor_tensor(
                    out=hd_row[:], in0=ps[:, 2 * H:], in1=bb_sb[li][:],
                    op=ALU.add)
                nc.gpsimd.dma_start(hd_dram[t * 128:(t + 1) * 128, :],
                                    hd_row[:])
                if t < HTILES:
                    nc.gpsimd.dma_start(
                        hsag_in_l[li][0][t * 128:(t + 1) * 128, :],
                        hs_row[:])
                else:
                    tb = t - HTILES
                    nc.gpsimd.dma_start(
                        hsag_in_l[li][1][tb * 128:(tb + 1) * 128, :],
                        hs_row[:])

            # ============ embed fused with layer-0 H ============
            UB = 4               # 128-node tiles per superblock
            with (
                tc.tile_pool(name="emb", bufs=3) as ep,
                tc.tile_pool(name="embp", bufs=2, space="PSUM") as epp,
                tc.tile_pool(name="embp2", bufs=2, space="PSUM") as epp2,
            ):
                for t0 in range(0, NB, UB):
                    nt = min(UB, NB - t0)
                    w = nt * 128
                    nftb = ep.tile([128, UB * 128], BF16, tag="nftb")
                    nc.sync.dma_start(nftb[:ND, :w],
                                       nfT_d[:, t0 * 128:t0 * 128 + w])
                    ps = epp.tile([128, UB * 128], F32, space="PSUM",
                                  tag="embp")
                    nc.tensor.matmul(ps[:, :w], lhsT=ew_sb[:ND, :],
                                     rhs=nftb[:ND, :w], start=True, stop=True)
                    h0 = ep.tile([128, UB * 128], BF16, tag="h0")
                    nc.vector.scalar_tensor_tensor(
                        out=h0[:, :w], in0=ps[:, :w], scalar=1.0,
                        in1=eb_sb[:].to_broadcast([128, w]),
                        op0=ALU.mult, op1=ALU.add)
                    nc.sync.dma_start(hT_dram[:, t0 * 128:t0 * 128 + w],
                                      h0[:, :w])
                    for ti in range(nt):
                        t = t0 + ti
                        ps2 = epp2.tile([128, 4 * H], F32, space="PSUM",
                                        tag="hsd")
                        nc.tensor.matmul(ps2[:],
                                         lhsT=h0[:, ti * 128:(ti + 1) * 128],
                                         rhs=wsd_sb[0][:],
                                         start=True, stop=True)
                        hs_hd_from_psum(ep, ps2, t, 0)
                    if t0 < HTILES <= t0 + nt:
                        nc.gpsimd.collective_compute(
                            "AllGather", ALU.bypass, replica_groups=RG,
                            ins=[hsag_in_l[0][0][:]],
                            outs=[hs_full_l[0][0][:]])
            nc.gpsimd.collective_compute(
                "AllGather", ALU.bypass, replica_groups=RG,
                ins=[hsag_in_l[0][1][:]], outs=[hs_full_l[0][1][:]])

            # ============ layers ============
            g_pool_psum = None
            for li in range(NL):
                hs_full_a, hs_full_b = hs_full_l[li]
                stat_in = stat_in_l[li]
                stat_out = stat_out_l[li]
                # load merged rhs for this layer: hd 108-block tiles + we rows
                nc.gpsimd.dma_start(
                    hdwe_sb[BS:BS + ED, :], weB_d[li][:, :])
                for b8 in range(NB8):
                    nc.gpsimd.dma_start(
                        hdwe_sb[:BS, b8 * 2 * H:(b8 + 1) * 2 * H],
                        hd_dram[b8 * BS:(b8 + 1) * BS, :])
                # ---- phase A: edges (bucket-major chunks) ----
                with (
                    tc.tile_pool(name="pg", bufs=2) as gp,
                    tc.tile_pool(name="pa", bufs=2) as ap,
                    tc.tile_pool(name="pde", bufs=3) as dp,
                    tc.tile_pool(name="pz", bufs=2, space="PSUM") as zp,
                    tc.tile_pool(name="pagg", bufs=2, space="PSUM") as agp,
                    tc.tile_pool(name="pst", bufs=1) as stp,
                ):
                    stats1 = stp.tile([128, NB8], F32, tag="st1")
                    stats2 = stp.tile([128, NB8], F32, tag="st2")
                    agg_ps = None
                    hcol0 = 0
                    for ci, (k, gc0, cgn) in enumerate(chunks):
                        q = cgn * 128
                        six = gp.tile([128, CHUNK_G * 8], mybir.dt.int16,
                                      tag="six")
                        nc.sync.dma_start(
                            six[:, :q // 16],
                            hsix_d[:, hcol0:hcol0 + q // 16])
                        hcol0 += q // 16
                        hsg = gp.tile([128, CHUNK_G * 2 * H], BF16, tag="hsg")
                        tab = hs_full_a if k < 2 else hs_full_b
                        lo = (k % 2) * BK
                        hi = lo + BK
                        nc.gpsimd.dma_gather(
                            out_ap=hsg[:, :cgn * 2 * H].rearrange(
                                "p (j c) -> p j c", c=2 * H),
                            in_ap=tab[lo:hi, :],
                            idxs_ap=six[:, :q // 16],
                            num_idxs=q, num_idxs_reg=q,
                            elem_size=2 * H, single_packet=False,
                            queue_num=ci % 2)
                        # ---- supertiles within chunk ----
                        for st0 in range(0, cgn, SUPER):
                            ns = min(SUPER, cgn - st0)
                            gbase = gc0 + st0
                            DES = dp.tile([128, SUPER * 256], BF16,
                                          tag="DES")
                            nc.sync.dma_start(
                                DES[:, :ns * 256],
                                DES_d[:, gbase * 256:(gbase + ns) * 256])
                            z = zp.tile([128, SUPER * 2 * H], F32,
                                        space="PSUM", tag="z")
                            for j in range(ns):
                                b_j = grp_kb[gbase + j][1]
                                nc.tensor.matmul(
                                    z[:, j * 2 * H:(j + 1) * 2 * H],
                                    lhsT=DES[:, j * 256:j * 256 + 128],
                                    rhs=hdwe_sb[:, b_j * 2 * H:(b_j + 1) * 2 * H],
                                    start=True, stop=True)
                            # inject gathered hs on DVE: zf = z + hsg
                            zf = ap.tile([128, SUPER * 2 * H], F32, tag="zf")
                            nc.vector.tensor_tensor(
                                out=zf[:, :ns * 2 * H], in0=z[:, :ns * 2 * H],
                                in1=hsg[:, (st0) * 2 * H:(st0 + ns) * 2 * H],
                                op=ALU.add)
                            # msg = sigmoid(zg)*softplus(zf'); zg' = -zg folded
                            a_t = ap.tile([128, SUPER * 2 * H], BF16, tag="a")
                            nc.scalar.activation(a_t[:, :ns * 2 * H],
                                                 zf[:, :ns * 2 * H], AF.Exp)
                            bd = ap.tile([128, SUPER * 2 * H], F32, tag="bd")
                            nc.scalar.activation(bd[:, :ns * 2 * H],
                                                 a_t[:, :ns * 2 * H], AF.Ln,
                                                 bias=1.0)
                            bd3 = bd[:, :ns * 2 * H].rearrange(
                                "p (g c) -> p g c", c=2 * H)
                            e_t = ap.tile([128, SUPER * H], BF16, tag="e")
                            e3 = e_t[:, :ns * H].rearrange(
                                "p (g c) -> p g c", c=H)
                            nc.scalar.activation(e3, bd3[:, :, 0:H], AF.Exp,
                                                 scale=-1.0)
                            msg = ap.tile([128, SUPER * H], BF16, tag="msg")
                            msg3 = msg[:, :ns * H].rearrange(
                                "p (g c) -> p g c", c=H)
                            nc.vector.tensor_tensor(
                                out=msg3, in0=e3, in1=bd3[:, :, H:2 * H],
                                op=ALU.mult)

                            for j in range(ns):
                                g = gbase + j
                                b_j = grp_kb[g][1]
                                if g in run_start:
                                    agg_ps = agp.tile([128, 128], F32,
                                                      space="PSUM",
                                                      tag="aggps")
                                    _, run_b, run_ng = run_start[g]
                                    run_end = g + run_ng - 1
                                nc.tensor.matmul(
                                    agg_ps[:, :BS],
                                    lhsT=msg[:, j * H:(j + 1) * H],
                                    rhs=DES[:, j * 256 + 128:
                                            j * 256 + 128 + BS],
                                    start=(g in run_start),
                                    stop=(g == run_end))
                                if g == run_end:
                                    asl = agg_sb[:, run_b * BS:
                                                 (run_b + 1) * BS]
                                    g0r = g - run_ng + 1
                                    if g0r in first_run_g0:
                                        nc.vector.tensor_copy(asl, agg_ps[:, :BS])
                                    else:
                                        nc.vector.tensor_tensor(
                                            out=asl, in0=asl,
                                            in1=agg_ps[:, :BS], op=ALU.add)
                                    if g0r in last_run_g0:
                                        sq = ap.tile([128, BS], F32, tag="sq")
                                        nc.vector.tensor_tensor(
                                            out=sq[:], in0=asl, in1=asl,
                                            op=ALU.mult)
                                        nc.vector.tensor_reduce(
                                            stats1[:, run_b:run_b + 1], asl,
                                            axis=mybir.AxisListType.X,
                                            op=ALU.add)
                                        nc.vector.tensor_reduce(
                                            stats2[:, run_b:run_b + 1], sq[:],
                                            axis=mybir.AxisListType.X,
                                            op=ALU.add)

                    stt = stp.tile([H, 2], F32, tag="stt")
                    nc.vector.tensor_reduce(stt[:, 0:1], stats1[:],
                                            axis=mybir.AxisListType.X,
                                            op=ALU.add)
                    nc.vector.tensor_reduce(stt[:, 1:2], stats2[:],
                                            axis=mybir.AxisListType.X,
                                            op=ALU.add)
                    nc.sync.dma_start(stat_in[:], stt[:])
                nc.gpsimd.collective_compute(
                    "AllReduce", ALU.add, replica_groups=RG,
                    ins=[stat_in[:]], outs=[stat_out[:]])

                # ---- phase S: bn scale/shift ----
                with tc.tile_pool(name="ps2", bufs=1) as sp2:
                    stf = sp2.tile([H, 2], F32, tag="stf")
                    nc.sync.dma_start(stf[:], stat_out[:])
                    mv = sp2.tile([H, 4], F32, tag="mv")
                    nc.vector.tensor_scalar_mul(mv[:, 0:1], stf[:, 0:1], 1.0 / N)
                    nc.vector.tensor_scalar_mul(mv[:, 1:2], stf[:, 1:2], 1.0 / N)
                    nc.vector.tensor_tensor(out=mv[:, 3:4], in0=mv[:, 0:1],
                                            in1=mv[:, 0:1], op=ALU.mult)
                    nc.vector.tensor_tensor(out=mv[:, 2:3], in0=mv[:, 1:2],
                                            in1=mv[:, 3:4], op=ALU.subtract)
                    nc.vector.tensor_scalar_add(mv[:, 2:3], mv[:, 2:3], EPS)
                    lnv = sp2.tile([H, 1], F32, tag="lnv")
                    nc.scalar.activation(lnv[:], mv[:, 2:3], AF.Ln)
                    nc.vector.tensor_scalar_mul(lnv[:], lnv[:], -0.5)
                    rs = sp2.tile([H, 1], F32, tag="rs")
                    nc.scalar.activation(rs[:], lnv[:], AF.Exp)
                    nc.vector.tensor_tensor(out=scsh_sb[:, 0:1], in0=rs[:],
                                            in1=bng_sb[li][:], op=ALU.mult)
                    nc.vector.tensor_tensor(out=mv[:, 3:4], in0=mv[:, 0:1],
                                            in1=scsh_sb[:, 0:1], op=ALU.mult)
                    nc.vector.tensor_tensor(out=scsh_sb[:, 1:2],
                                            in0=bnb_sb[li][:], in1=mv[:, 3:4],
                                            op=ALU.subtract)

                # ---- fused phase U (+ next-layer H, or pooling on last) ----
                last = li == NL - 1
                with (
                    tc.tile_pool(name="pu", bufs=4) as up,
                    tc.tile_pool(name="pup", bufs=2, space="PSUM") as upp,
                    tc.tile_pool(name="pug", bufs=1, space="PSUM") as ugp,
                ):
                    if last:
                        g_pool_psum = ugp.tile([B, H], F32, space="PSUM",
                                               tag="gpool")
                    UB = 4           # 128-node tiles per superblock
                    for t0 in range(0, NB, UB):
                        nt = min(UB, NB - t0)
                        w = nt * 128
                        ht = up.tile([128, UB * 128], BF16, tag="ht")
                        nc.sync.dma_start(
                            ht[:, :w], hT_dram[:, t0 * 128:t0 * 128 + w])
                        t1 = up.tile([128, UB * 128], F32, tag="t1")
                        nc.vector.scalar_tensor_tensor(
                            out=t1[:, :w],
                            in0=agg_sb[:, t0 * 128:t0 * 128 + w],
                            scalar=scsh_sb[:, 0:1],
                            in1=scsh_sb[:, 1:2].to_broadcast([128, w]),
                            op0=ALU.mult, op1=ALU.add)
                        t2 = up.tile([128, UB * 128], F32, tag="t2")
                        nc.vector.tensor_tensor(out=t2[:, :w], in0=t1[:, :w],
                                                in1=ht[:, :w], op=ALU.add)
                        hnew = up.tile([128, UB * 128], BF16, tag="hnew")
                        nc.vector.tensor_scalar_max(hnew[:, :w], t2[:, :w],
                                                    0.0)
                        if not last:
                            nc.sync.dma_start(
                                hT_dram[:, t0 * 128:t0 * 128 + w],
                                hnew[:, :w])
                        for ti in range(nt):
                            t = t0 + ti
                            hv = hnew[:, ti * 128:(ti + 1) * 128]
                            if not last:
                                ps2 = upp.tile([128, 4 * H], F32,
                                               space="PSUM", tag="hsd")
                                nc.tensor.matmul(ps2[:], lhsT=hv,
                                                 rhs=wsd_sb[li + 1][:],
                                                 start=True, stop=True)
                                hs_hd_from_psum(up, ps2, t, li + 1)
                            else:
                                tp = upp.tile([128, 128], BF16, space="PSUM",
                                              tag="tp")
                                nc.tensor.transpose(out=tp[:], in_=hv,
                                                    identity=id_bf[:])
                                hbk = up.tile([128, 128], BF16, tag="hbk")
                                nc.vector.tensor_copy(hbk[:], tp[:])
                                Bt = up.tile([128, B], BF16, tag="Bt")
                                nc.sync.dma_start(Bt[:],
                                                  Bt_d[:, t * B:(t + 1) * B])
                                nc.tensor.matmul(g_pool_psum[:], lhsT=Bt[:],
                                                 rhs=hbk[:], start=(t == 0),
                                                 stop=(t == NB - 1))
                        if not last and t0 < HTILES <= t0 + nt:
                            nc.gpsimd.collective_compute(
                                "AllGather", ALU.bypass, replica_groups=RG,
                                ins=[hsag_in_l[li + 1][0][:]],
                                outs=[hs_full_l[li + 1][0][:]])
                    if last:
                        gsb = up.tile([B, H], F32, tag="gsb")
                        nc.vector.tensor_copy(gsb[:], g_pool_psum[:])
                        nc.sync.dma_start(g_in[:], gsb[:])
                if not last:
                    nc.gpsimd.collective_compute(
                        "AllGather", ALU.bypass, replica_groups=RG,
                        ins=[hsag_in_l[li + 1][1][:]],
                        outs=[hs_full_l[li + 1][1][:]])

            # ============ head ============
            nc.gpsimd.collective_compute(
                "AllReduce", ALU.add, replica_groups=RG,
                ins=[g_in[:]], outs=[g_out[:]])
            with (
                tc.tile_pool(name="hd2", bufs=1) as hp2,
                tc.tile_pool(name="hdp2", bufs=2, space="PSUM") as hpp2,
            ):
                gsum = hp2.tile([B, H], F32, tag="gsum")
                nc.sync.dma_start(gsum[:], g_out[:])
                gmean = hp2.tile([B, H], F32, tag="gmean")
                nc.vector.tensor_scalar_mul(gmean[:], gsum[:], cinv_sb[:, 0:1])
                tps = hpp2.tile([H, B], F32, space="PSUM", tag="tps")
                nc.tensor.transpose(out=tps[:], in_=gmean[:],
                                    identity=id_f32[:B, :B])
                gT = hp2.tile([H, B], F32, tag="gT")
                nc.vector.tensor_copy(gT[:], tps[:])
                x1p = hpp2.tile([H, B], F32, space="PSUM", tag="x1p")
                nc.tensor.matmul(x1p[:], lhsT=fc1w_sb[:], rhs=gT[:],
                                 start=True, stop=True)
                x1 = hp2.tile([H, B], F32, tag="x1")
                nc.vector.scalar_tensor_tensor(
                    out=x1[:], in0=x1p[:], scalar=1.0,
                    in1=fc1b_sb[:].to_broadcast([H, B]),
                    op0=ALU.mult, op1=ALU.add)
                sc2 = hp2.tile([H, 6], F32, tag="sc2")
                nc.vector.tensor_reduce(sc2[:, 0:1], x1[:],
                                        axis=mybir.AxisListType.X, op=ALU.add)
                nc.vector.tensor_scalar_mul(sc2[:, 1:2], sc2[:, 0:1], 1.0 / B)
                xsq = hp2.tile([H, B], F32, tag="xsq")
                nc.vector.tensor_tensor(out=xsq[:], in0=x1[:], in1=x1[:],
                                        op=ALU.mult)
                nc.vector.tensor_reduce(sc2[:, 2:3], xsq[:],
                                        axis=mybir.AxisListType.X, op=ALU.add)
                nc.vector.tensor_scalar_mul(sc2[:, 2:3], sc2[:, 2:3], 1.0 / B)
                nc.vector.tensor_tensor(out=sc2[:, 3:4], in0=sc2[:, 1:2],
                                        in1=sc2[:, 1:2], op=ALU.mult)
                nc.vector.tensor_tensor(out=sc2[:, 3:4], in0=sc2[:, 2:3],
                                        in1=sc2[:, 3:4], op=ALU.subtract)
                nc.vector.tensor_scalar_add(sc2[:, 3:4], sc2[:, 3:4], EPS)
                lnv2 = hp2.tile([H, 1], F32, tag="lnv2")
                nc.scalar.activation(lnv2[:], sc2[:, 3:4], AF.Ln)
                nc.vector.tensor_scalar_mul(lnv2[:], lnv2[:], -0.5)
                rs2 = hp2.tile([H, 1], F32, tag="rs2")
                nc.scalar.activation(rs2[:], lnv2[:], AF.Exp)
                nc.vector.tensor_tensor(out=sc2[:, 4:5], in0=rs2[:],
                                        in1=fcg_sb[:], op=ALU.mult)
                nc.vector.tensor_tensor(out=sc2[:, 3:4], in0=sc2[:, 1:2],
                                        in1=sc2[:, 4:5], op=ALU.mult)
                nc.vector.tensor_tensor(out=sc2[:, 5:6], in0=fcb_sb[:],
                                        in1=sc2[:, 3:4], op=ALU.subtract)
                x2 = hp2.tile([H, B], F32, tag="x2")
                nc.vector.scalar_tensor_tensor(
                    out=x2[:], in0=x1[:], scalar=sc2[:, 4:5],
                    in1=sc2[:, 5:6].to_broadcast([H, B]),
                    op0=ALU.mult, op1=ALU.add)
                x2r = hp2.tile([H, B], F32, tag="x2r")
                nc.vector.tensor_scalar_max(x2r[:], x2[:], 0.0)
                yp = hpp2.tile([1, B], F32, space="PSUM", tag="yp")
                nc.tensor.matmul(yp[:], lhsT=outw_sb[:], rhs=x2r[:],
                                 start=True, stop=True)
                ysb = hp2.tile([1, B], F32, tag="ysb")
                nc.vector.tensor_scalar_add(ysb[:], yp[:], meta["out_b"])
                nc.sync.dma_start(out_d[:].rearrange("b o -> o b"), ysb[:])

    return nc


# --------------------------------------------------------------------------
def run(inputs, cfg=None, debug=False):
    cfg = cfg or DEFAULT_CFG
    meta, in_maps = _preprocess(cfg, inputs)
    nc = _build(cfg, meta, debug=debug)
    nc.finalize()
    res = run_bass_kernel_spmd(nc, in_maps, core_ids=list(range(NCORES)))
    return np.asarray(res.results[0]["out"], np.float32)


def kernel(**inputs):
    return run(inputs, DEFAULT_CFG)


# revision 34
# speedup vs baseline: 1.5869x; 1.5869x over previous
"""CGCNN message-passing kernel for 8 Trainium2 NeuronCores.

Strategy (v3 — merged one-hot+edge-feat matmul, 108-node dst blocks):
  - Nodes partitioned contiguously across 8 cores (NPC nodes each, padded).
  - Edges sharded by dst owner, grouped by 108-node dst block, laid out
    BUCKET-MAJOR: for src-bucket k (32768-node windows, int16 gather range),
    for dst-block b, quota[b,k] slots (multiple of 128, max over cores so a
    single SPMD program fits all cores). Groups of 128 slots are
    (bucket, block)-pure.
  - hs gather: per (bucket, chunk of CHUNK_G groups) dma_gather on 2
    alternating SWDGE queues (~5.5 ns/idx measured; desc-gen is the cost).
  - z per group in ONE matmul: lhsT = DE_g = [one-hot dst (108 rows);
    ef^T (20 rows)], rhs = hdwe_b = [hd block (108 rows); we (20 rows)]:
    z = hd[dst] + ef@we in a single K=128 pass. 108 + ED(=20) = 128 exactly.
  - Gathered hs injected by DVE (z_sbuf = z_psum + hsg), not a PE matmul.
  - msg = sigmoid(zg)*softplus(zf) with the gate sign folded into weights
    (zg' = -zg): a = exp(z'), bd = ln(1+a), e = exp(-b_gate), msg = e*d.
  - Aggregation: agg^T[feat, nodes] += msg_j^T @ S_j (N=108) in PSUM per
    (bucket, block) run; runs flushed (copy/add) into an SBUF-resident
    agg accumulator; BN stats reduced from SBUF at each block's last run.
  - Fused U+H pass per 128-node tile: hnew = relu(h + bn(agg)) then
    hs/hd = hnew @ [wsrc|wdst] in one pass (embed fused the same way for
    layer 0). hd goes to DRAM and is reloaded 108-block-tiled next layer.
  - A post-pass rewrites compiler-inserted ACT_TABLE_LOADs to the shared
    natural_log_exp table set and dedups them.
  - Mean-pool via one-hot matmul + AllReduce; FC head replicated.

kernel(**inputs) -> np.ndarray (B, 1) float32. Self-contained (shapes
hardcoded below; no file reads).
"""
import sys

sys.path.insert(0, "/opt/trn_rl_repo")

import numpy as np
from ml_dtypes import bfloat16

import concourse.bacc as bacc
import concourse.bass as bass
import concourse.mybir as mybir
import concourse.tile as tile
from concourse.bass_utils import run_bass_kernel_spmd
from concourse.masks import make_identity

F32 = mybir.dt.float32
BF16 = mybir.dt.bfloat16
I32 = mybir.dt.int32
AF = mybir.ActivationFunctionType
ALU = mybir.AluOpType

NCORES = 8
GROUP = 128
SUPER = 6            # groups per supertile: z-PSUM = 3 banks, double buffered
CHUNK_G = 24         # groups per dma_gather call (must be multiple of SUPER)
BS = 108             # dst-block size; BS + ED = 128 (merged matmul rows)
EPS = 1e-5

_NL_EXP_SET_ID = 6


def _patch_act_table_loads():
    if getattr(bacc.Bacc, "_atl_patched", False):
        return
    orig = bacc.Bacc.insert_act_table_loads

    def patched(self):
        orig(self)
        for blk in self.main_func.blocks:
            first_seen = False
            drop = []
            for ins in blk.instructions:
                if not isinstance(ins, mybir.InstLoadActFuncSet):
                    continue
                ins.act_func_set_id = _NL_EXP_SET_ID
                si = getattr(ins, "sync_info", None)
                has_sync = si is not None and (
                    len(getattr(si, "on_wait", [])) > 0
                    or len(getattr(si, "on_update", [])) > 0)
                if not first_seen or has_sync:
                    first_seen = True
                    continue
                drop.append(ins)
            if drop:
                dropset = set(map(id, drop))
                blk.instructions[:] = [
                    i for i in blk.instructions if id(i) not in dropset]

    bacc.Bacc.insert_act_table_loads = patched
    bacc.Bacc._atl_patched = True


_patch_act_table_loads()


class Cfg:
    def __init__(self, N, E, B, ND, ED, H, NL):
        self.N, self.E, self.B = N, E, B
        self.ND, self.ED, self.H, self.NL = ND, ED, H, NL
        per = (N + NCORES - 1) // NCORES
        self.NPC = (per + 127) // 128 * 128
        self.NB = self.NPC // 128          # 128-node tiles (U/H passes)
        self.NB8 = -(-self.NPC // BS)      # 108-node dst blocks (edge phase)
        self.NPAD = self.NPC * NCORES


DEFAULT_CFG = Cfg(N=100000, E=1600000, B=128, ND=91, ED=20, H=128, NL=3)


# --------------------------------------------------------------------------
# Host-side preprocessing
# --------------------------------------------------------------------------
def _preprocess(cfg, inputs):
    N, B, H, ED, ND, NL = cfg.N, cfg.B, cfg.H, cfg.ED, cfg.ND, cfg.NL
    NPC, NB8 = cfg.NPC, cfg.NB8

    src = np.asarray(inputs["edge_index"][0], np.int64)
    dst = np.asarray(inputs["edge_index"][1], np.int64)
    ef32 = np.asarray(inputs["edge_feats"], np.float32)
    batch_np = np.asarray(inputs["batch"], np.int64)

    order = np.argsort(dst, kind="stable")
    src_s, dst_s, e_s = src[order], dst[order], order

    # hs tables are AllGathered in TWO halves (overlap with the update
    # pass): node n -> (core c, local r); half = r >= NPC//2; table row =
    # c*(NPC//2) + r%(NPC//2). Each half-table (NPAD//2 rows) splits into
    # 2 gather buckets of BK rows (int16 range).
    NBK = 4
    HALF = NPC // 2
    BK = NCORES * HALF // 2          # 25088 rows/bucket

    def table_row_bucket(n):
        c = n // NPC
        r = n - c * NPC
        half = (r >= HALF).astype(np.int64)
        row = c * HALF + (r - half * HALF)
        k = half * 2 + row // BK
        rel = row - (row // BK) * BK
        return k, rel

    starts = np.searchsorted(dst_s, np.arange(NCORES) * NPC)
    ends = np.searchsorted(dst_s, np.arange(NCORES) * NPC + NPC)

    src_k, src_rel = table_row_bucket(src_s)

    # per-core, per-block, per-bucket edge counts
    cnt = np.zeros((NCORES, NB8, NBK), np.int64)
    core_block = []
    for c in range(NCORES):
        lo, hi = int(starts[c]), int(ends[c])
        cdstl = dst_s[lo:hi] - c * NPC
        cblk = cdstl // BS
        bstart = np.searchsorted(cblk, np.arange(NB8))
        bend = np.searchsorted(cblk, np.arange(NB8) + 1)
        core_block.append((lo, bstart, bend))
        for b in range(NB8):
            bk = src_k[lo + bstart[b]:lo + bend[b]]
            cnt[c, b] = np.bincount(bk, minlength=NBK)

    # shared quotas: per (block, bucket), max over cores, rounded to x128
    quota = -(-cnt.max(axis=0) // GROUP) * GROUP          # [NB8, NBK]
    for b in range(NB8):
        if quota[b].sum() == 0:
            quota[b, 0] = GROUP
    G = int(quota.sum()) // GROUP

    # bucket-major group order: for k, for b, quota[b,k]//128 groups.
    grp_kb = []          # group -> (k, b)
    runs = []            # (k, b, g0, ng)
    for k in range(NBK):
        for b in range(NB8):
            ng = int(quota[b, k]) // GROUP
            if ng == 0:
                continue
            runs.append((k, b, len(grp_kb), ng))
            grp_kb += [(k, b)] * ng
    assert len(grp_kb) == G
    run_of_group_start = {g0: (k, b, ng) for (k, b, g0, ng) in runs}
    firstg, lastg = {}, {}
    for (k, b, g0, ng) in runs:
        if b not in firstg:
            firstg[b] = g0
        lastg[b] = g0
    first_run_g0 = set(firstg.values())
    last_run_g0 = set(lastg.values())

    # chunks: list of (k, g0, ng) with ng <= CHUNK_G, bucket-pure
    chunks = []
    gk0 = 0
    for k in range(NBK):
        ngk = sum(int(quota[b, k]) // GROUP for b in range(NB8))
        g = 0
        while g < ngk:
            cgn = min(CHUNK_G, ngk - g)
            chunks.append((k, gk0 + g, cgn))
            g += cgn
        gk0 += ngk

    conv_b = np.asarray(inputs["conv_b"], np.float32).copy()
    counts = np.maximum(np.bincount(batch_np, minlength=B), 1.0)
    counts_inv = (1.0 / counts).astype(np.float32).reshape(B, 1)
    wsd = [np.concatenate([np.asarray(inputs["conv_wsrc"][i], np.float32),
                           np.asarray(inputs["conv_wdst"][i], np.float32)],
                          axis=1) for i in range(NL)]             # [H, 4H]
    we_l = [np.asarray(inputs["conv_we"][i], np.float32).copy()
            for i in range(NL)]
    # fold the gate sign into the weights: zg' = -zg
    for i in range(NL):
        wsd[i][:, 0:H] *= -1.0          # wsrc gate cols
        wsd[i][:, 2 * H:3 * H] *= -1.0  # wdst gate cols
        we_l[i][:, 0:H] *= -1.0
        conv_b[i, 0:H] *= -1.0
    nf = np.asarray(inputs["node_feats"], np.float32)

    def wrap16(flat):
        n = flat.shape[0]
        w = flat.reshape(n // 16, 16).T
        return np.tile(w, (8, 1)).astype(np.int16)

    hs_cols = int(quota.sum()) // 16                      # total idx cols
    in_maps = []
    for c in range(NCORES):
        lo, bstart, bend = core_block[c]
        S = np.zeros((128, G * 128), np.float32)
        DE = np.zeros((128, G * 128), np.float32)
        slot_rel = np.zeros(G * 128, np.int64)     # in-bucket table offset
        for (k, b, g0, ngr) in runs:
            eb = slice(lo + bstart[b], lo + bend[b])
            eb_k = src_k[eb]
            eb_rel = src_rel[eb]
            eb_dstl = dst_s[eb] - c * NPC
            eb_e = e_s[eb]
            sel = eb_k == k
            erel, ed, ee = eb_rel[sel], eb_dstl[sel], eb_e[sel]
            q = ngr * 128
            n_e = erel.shape[0]
            assert n_e <= q, (c, b, k, n_e, q)
            off = g0 * 128
            slot_rel[off:off + n_e] = erel
            slot_rel[off + n_e:off + q] = 0            # pad: bucket base row
            ii = np.arange(n_e)
            p = ii % 128
            gg = g0 + ii // 128
            s_in_b = ed - b * BS
            cols = gg * 128 + p
            # one-hot scatter: S[p, g*128 + s], DE[s, g*128 + p]
            S[p, gg * 128 + s_in_b] = 1.0
            DE[s_in_b, gg * 128 + p] = 1.0
            DE[BS:BS + ED, :][:, gg * 128 + p] = ef32[ee].T

        hs_idx = np.zeros((128, hs_cols), np.int16)
        hcol = 0
        for (k, g0, cgn) in chunks:
            q = cgn * 128
            rel = slot_rel[g0 * 128:g0 * 128 + q].astype(np.int16)
            hs_idx[:, hcol:hcol + q // 16] = wrap16(rel)
            hcol += q // 16
        assert hcol == hs_cols

        NB = cfg.NB
        Bt = np.zeros((128, NB * B), np.float32)
        for t in range(NB):
            ids = np.arange(c * NPC + t * 128, c * NPC + t * 128 + 128)
            valid = ids < N
            gv = np.where(valid, batch_np[np.minimum(ids, N - 1)], 0)
            Bt[np.arange(128)[valid], t * B + gv[valid]] = 1.0

        nfT = np.zeros((ND, NPC), np.float32)  # cast to bf16 below
        n_real = min(NPC, max(0, N - c * NPC))
        if n_real > 0:
            nfT[:, :n_real] = nf[c * NPC:c * NPC + n_real].T

        m = {
            "nfT": nfT.astype(bfloat16),
            "hs_idx": hs_idx,
            "S": S.astype(bfloat16),
            "DE": DE.astype(bfloat16),
            "Btiles": Bt.astype(bfloat16),
            "embed_w": np.asarray(inputs["embed_w"], np.float32),
            "embed_b": np.asarray(inputs["embed_b"], np.float32).reshape(H, 1),
            "counts_inv": counts_inv,
            "fc1_w": np.asarray(inputs["fc1_w"], np.float32),
            "fc1_b": np.asarray(inputs["fc1_b"], np.float32).reshape(H, 1),
            "fc_g": np.asarray(inputs["fc_bn_gamma"], np.float32).reshape(H, 1),
            "fc_be": np.asarray(inputs["fc_bn_beta"], np.float32).reshape(H, 1),
            "out_w": np.asarray(inputs["out_w"], np.float32).reshape(H, 1),
        }
        for i in range(NL):
            m[f"wsd{i}"] = wsd[i]
            # we tiled per dst block for the merged rhs (rows BS..BS+ED)
            m[f"weB{i}"] = np.tile(we_l[i].astype(np.float32),
                                   (1, NB8)).astype(bfloat16)  # [ED, NB8*2H]
            m[f"biasb{i}"] = np.tile(conv_b[i][None, :], (128, 1))
            m[f"bn_g{i}"] = np.asarray(inputs["bn_gamma"][i], np.float32).reshape(H, 1)
            m[f"bn_b{i}"] = np.asarray(inputs["bn_beta"][i], np.float32).reshape(H, 1)
        in_maps.append(m)

    meta = dict(G=G, BK=BK, NBK=NBK,
                grp_kb=grp_kb, chunks=chunks,
                run_start=run_of_group_start,
                first_run_g0=first_run_g0, last_run_g0=last_run_g0,
                out_b=float(np.asarray(inputs["out_b"]).reshape(-1)[0]))
    return meta, in_maps


# --------------------------------------------------------------------------
# Device program
# --------------------------------------------------------------------------
def _build(cfg, meta, debug=False):
    N, B, H, ED, ND, NL = cfg.N, cfg.B, cfg.H, cfg.ED, cfg.ND, cfg.NL
    NPC, NB, NB8, NPAD = cfg.NPC, cfg.NB, cfg.NB8, cfg.NPAD
    G = meta["G"]
    BK, NBK = meta["BK"], meta["NBK"]
    grp_kb = meta["grp_kb"]
    chunks = meta["chunks"]
    run_start = meta["run_start"]
    first_run_g0 = meta["first_run_g0"]
    last_run_g0 = meta["last_run_g0"]
    hs_cols = G * 128 // 16
    RG = [list(range(NCORES))]

    nc = bacc.Bacc("TRN2", target_bir_lowering=False, debug=False,
                   num_devices=NCORES, num_swdge_queues=2)

    def inp(name, shape, dt=F32):
        return nc.dram_tensor(name, shape, dt, kind="ExternalInput")

    nfT_d = inp("nfT", [ND, NPC], BF16)
    hsix_d = inp("hs_idx", [128, hs_cols], mybir.dt.int16)
    S_d = inp("S", [128, G * 128], BF16)
    DE_d = inp("DE", [128, G * 128], BF16)
    Bt_d = inp("Btiles", [128, NB * B], BF16)
    ew_d = inp("embed_w", [ND, H])
    eb_d = inp("embed_b", [H, 1])
    cinv_d = inp("counts_inv", [B, 1])
    fc1w_d = inp("fc1_w", [H, H])
    fc1b_d = inp("fc1_b", [H, 1])
    fcg_d = inp("fc_g", [H, 1])
    fcb_d = inp("fc_be", [H, 1])
    outw_d = inp("out_w", [H, 1])
    wsd_d = [inp(f"wsd{i}", [H, 4 * H]) for i in range(NL)]
    weB_d = [inp(f"weB{i}", [ED, NB8 * 2 * H], BF16) for i in range(NL)]
    bb_d = [inp(f"biasb{i}", [128, 2 * H]) for i in range(NL)]
    bng_d = [inp(f"bn_g{i}", [H, 1]) for i in range(NL)]
    bnb_d = [inp(f"bn_b{i}", [H, 1]) for i in range(NL)]
    out_d = nc.dram_tensor("out", [B, 1], F32, kind="ExternalOutput")

    with tile.TileContext(nc) as tc:
        with (
            tc.tile_pool(name="const", bufs=1) as cp,
            tc.tile_pool(name="dram", bufs=1, space="DRAM") as dr,
        ):
            HALF = NPC // 2
            HTILES = HALF // 128     # 128-node tiles per half
            hT_dram = dr.tile([128, NPC], BF16, tag="hT")
            # 108-tiled hd table; rows NPC..NB8*BS are padding, zeroed once
            # (one-hot matmul multiplies them by 0 — must be finite)
            hd_dram = dr.tile([NB8 * BS, 2 * H], BF16, tag="hdd")
            hsag_in_l, hs_full_l, stat_in_l, stat_out_l = [], [], [], []
            for i in range(NL):
                ha = dr.tile([HALF, 2 * H], BF16, tag=f"hsina{i}",
                             name=f"hsina{i}")
                hb = dr.tile([HALF, 2 * H], BF16, tag=f"hsinb{i}",
                             name=f"hsinb{i}")
                hsag_in_l.append((ha, hb))
                fa = dr.tile([NPAD // 2, 2 * H], BF16, tag=f"hsfa{i}",
                             addr_space="Shared", name=f"hsfa{i}")
                fb = dr.tile([NPAD // 2, 2 * H], BF16, tag=f"hsfb{i}",
                             addr_space="Shared", name=f"hsfb{i}")
                hs_full_l.append((fa, fb))
                sti_t = dr.tile([H, 2], F32, tag=f"stin{i}", name=f"stin{i}")
                stat_in_l.append(sti_t)
                sto_t = dr.tile([H, 2], F32, tag=f"stout{i}", name=f"stout{i}")
                stat_out_l.append(sto_t)
            g_in = dr.tile([B, H], F32, tag="gin")
            g_out = dr.tile([B, H], F32, tag="gout")

            # ---- resident constants ----
            ew_sb = cp.tile([128, H], BF16, tag="ew")
            nc.gpsimd.dma_start(ew_sb[:ND, :], ew_d[:, :])
            eb_sb = cp.tile([H, 1], F32, tag="eb")
            nc.sync.dma_start(eb_sb[:, :], eb_d[:, :])
            wsd_sb, bb_sb, bng_sb, bnb_sb = [], [], [], []
            for i in range(NL):
                w = cp.tile([H, 4 * H], BF16, tag=f"wsd{i}")
                nc.gpsimd.dma_start(w[:, :], wsd_d[i][:, :])
                wsd_sb.append(w)
                w = cp.tile([128, 2 * H], F32, tag=f"bb{i}")
                nc.sync.dma_start(w[:, :], bb_d[i][:, :])
                bb_sb.append(w)
                w = cp.tile([H, 1], F32, tag=f"bng{i}")
                nc.sync.dma_start(w[:, :], bng_d[i][:, :])
                bng_sb.append(w)
                w = cp.tile([H, 1], F32, tag=f"bnb{i}")
                nc.sync.dma_start(w[:, :], bnb_d[i][:, :])
                bnb_sb.append(w)
            cinv_sb = cp.tile([B, 1], F32, tag="cinv")
            nc.sync.dma_start(cinv_sb[:, :], cinv_d[:, :])
            fc1w_sb = cp.tile([H, H], F32, tag="fc1w")
            nc.sync.dma_start(fc1w_sb[:, :], fc1w_d[:, :])
            fc1b_sb = cp.tile([H, 1], F32, tag="fc1b")
            nc.sync.dma_start(fc1b_sb[:, :], fc1b_d[:, :])
            fcg_sb = cp.tile([H, 1], F32, tag="fcg")
            nc.sync.dma_start(fcg_sb[:, :], fcg_d[:, :])
            fcb_sb = cp.tile([H, 1], F32, tag="fcb")
            nc.sync.dma_start(fcb_sb[:, :], fcb_d[:, :])
            outw_sb = cp.tile([H, 1], F32, tag="outw")
            nc.sync.dma_start(outw_sb[:, :], outw_d[:, :])
            id_bf = cp.tile([128, 128], BF16, tag="idbf")
            make_identity(nc, id_bf[:])
            id_f32 = cp.tile([128, 128], F32, tag="idf32")
            make_identity(nc, id_f32[:])
            scsh_sb = cp.tile([H, 2], F32, tag="scsh")
            # merged rhs: rows 0..BS-1 hd per block, rows BS..127 we
            hdwe_sb = cp.tile([128, NB8 * 2 * H], BF16, tag="hdwe")
            # SBUF-resident agg accumulator [feat, padded local nodes]
            agg_sb = cp.tile([128, NB8 * BS], F32, tag="aggsb")
            zpad = cp.tile([128, 2 * H], BF16, tag="zpad")
            nc.vector.memset(zpad[:], 0.0)
            nc.sync.dma_start(hd_dram[NPC:NB8 * BS, :],
                              zpad[:NB8 * BS - NPC, :])

            def hs_hd_from_psum(pool, ps, t, li):
                """ps [128, 4H] = h@[wsrc|wdst]; emit hs row + hd to DRAM.
                Stores ride the gpsimd DMA queue (sync is the bottleneck in
                the fused update pass)."""
                hs_row = pool.tile([128, 2 * H], BF16, tag="hsr")
                nc.scalar.activation(hs_row[:], ps[:, :2 * H], AF.Copy)
                hd_row = pool.tile([128, 2 * H], BF16, tag="hdr")
                nc.vector.tensor_tensor(
                    out=hd_row[:], in0=ps[:, 2 * H:], in1=bb_sb[li][:],
                    op=ALU.add)
                nc.gpsimd.dma_start(hd_dram[t * 128:(t + 1) * 128, :],
                                    hd_row[:])
                if t < HTILES:
                    nc.gpsimd.dma_start(
                        hsag_in_l[li][0][t * 128:(t + 1) * 128, :],
                        hs_row[:])
                else:
                    tb = t - HTILES
                    nc.gpsimd.dma_start(
                        hsag_in_l[li][1][tb * 128:(tb + 1) * 128, :],
                        hs_row[:])

            # ============ embed fused with layer-0 H ============
            UB = 4               # 128-node tiles per superblock
            with (
                tc.tile_pool(name="emb", bufs=3) as ep,
                tc.tile_pool(name="embp", bufs=2, space="PSUM") as epp,
                tc.tile_pool(name="embp2", bufs=2, space="PSUM") as epp2,
            ):
                for t0 in range(0, NB, UB):
                    nt = min(UB, NB - t0)
                    w = nt * 128
                    nftb = ep.tile([128, UB * 128], BF16, tag="nftb")
                    nc.sync.dma_start(nftb[:ND, :w],
                                       nfT_d[:, t0 * 128:t0 * 128 + w])
                    ps = epp.tile([128, UB * 128], F32, space="PSUM",
                                  tag="embp")
                    nc.tensor.matmul(ps[:, :w], lhsT=ew_sb[:ND, :],
                                     rhs=nftb[:ND, :w], start=True, stop=True)
                    h0 = ep.tile([128, UB * 128], BF16, tag="h0")
                    nc.vector.scalar_tensor_tensor(
                        out=h0[:, :w], in0=ps[:, :w], scalar=1.0,
                        in1=eb_sb[:].to_broadcast([128, w]),
                        op0=ALU.mult, op1=ALU.add)
                    nc.sync.dma_start(hT_dram[:, t0 * 128:t0 * 128 + w],
                                      h0[:, :w])
                    for ti in range(nt):
                        t = t0 + ti
                        ps2 = epp2.tile([128, 4 * H], F32, space="PSUM",
                                        tag="hsd")
                        nc.tensor.matmul(ps2[:],
                                         lhsT=h0[:, ti * 128:(ti + 1) * 128],
                                         rhs=wsd_sb[0][:],
                                         start=True, stop=True)
                        hs_hd_from_psum(ep, ps2, t, 0)
                    if t0 < HTILES <= t0 + nt:
                        nc.gpsimd.collective_compute(
                            "AllGather", ALU.bypass, replica_groups=RG,
                            ins=[hsag_in_l[0][0][:]],
                            outs=[hs_full_l[0][0][:]])
            nc.gpsimd.collective_compute(
                "AllGather", ALU.bypass, replica_groups=RG,
                ins=[hsag_in_l[0][1][:]], outs=[hs_full_l[0][1][:]])

            # ============ layers ============
            g_pool_psum = None
            for li in range(NL):
                hs_full_a, hs_full_b = hs_full_l[li]
                stat_in = stat_in_l[li]
                stat_out = stat_out_l[li]
                # load merged rhs for this layer: hd 108-block tiles + we rows
                nc.gpsimd.dma_start(
                    hdwe_sb[BS:BS + ED, :], weB_d[li][:, :])
                for b8 in range(NB8):
                    nc.gpsimd.dma_start(
                        hdwe_sb[:BS, b8 * 2 * H:(b8 + 1) * 2 * H],
                        hd_dram[b8 * BS:(b8 + 1) * BS, :])
                # ---- phase A: edges (bucket-major chunks) ----
                with (
                    tc.tile_pool(name="pg", bufs=2) as gp,
                    tc.tile_pool(name="pa", bufs=2) as ap,
                    tc.tile_pool(name="pde", bufs=3) as dp,
                    tc.tile_pool(name="pz", bufs=2, space="PSUM") as zp,
                    tc.tile_pool(name="pagg", bufs=2, space="PSUM") as agp,
                    tc.tile_pool(name="pst", bufs=1) as stp,
                ):
                    stats1 = stp.tile([128, NB8], F32, tag="st1")
                    stats2 = stp.tile([128, NB8], F32, tag="st2")
                    agg_ps = None
                    hcol0 = 0
                    for ci, (k, gc0, cgn) in enumerate(chunks):
                        q = cgn * 128
                        six = gp.tile([128, CHUNK_G * 8], mybir.dt.int16,
                                      tag="six")
                        nc.sync.dma_start(
                            six[:, :q // 16],
                            hsix_d[:, hcol0:hcol0 + q // 16])
                        hcol0 += q // 16
                        hsg = gp.tile([128, CHUNK_G * 2 * H], BF16, tag="hsg")
                        tab = hs_full_a if k < 2 else hs_full_b
                        lo = (k % 2) * BK
                        hi = lo + BK
                        nc.gpsimd.dma_gather(
                            out_ap=hsg[:, :cgn * 2 * H].rearrange(
                                "p (j c) -> p j c", c=2 * H),
                            in_ap=tab[lo:hi, :],
                            idxs_ap=six[:, :q // 16],
                            num_idxs=q, num_idxs_reg=q,
                            elem_size=2 * H, single_packet=False,
                            queue_num=ci % 2)
                        # ---- supertiles within chunk ----
                        for st0 in range(0, cgn, SUPER):
                            ns = min(SUPER, cgn - st0)
                            gbase = gc0 + st0
                            DEsb = dp.tile([128, SUPER * 128], BF16,
                                           tag="DEsb")
                            nc.sync.dma_start(
                                DEsb[:, :ns * 128],
                                DE_d[:, gbase * 128:(gbase + ns) * 128])
                            Ssb = dp.tile([128, SUPER * 128], BF16,
                                          tag="Ssb")
                            nc.sync.dma_start(
                                Ssb[:, :ns * 128],
                                S_d[:, gbase * 128:(gbase + ns) * 128])
                            z = zp.tile([128, SUPER * 2 * H], F32,
                                        space="PSUM", tag="z")
                            for j in range(ns):
                                b_j = grp_kb[gbase + j][1]
                                nc.tensor.matmul(
                                    z[:, j * 2 * H:(j + 1) * 2 * H],
                                    lhsT=DEsb[:, j * 128:(j + 1) * 128],
                                    rhs=hdwe_sb[:, b_j * 2 * H:(b_j + 1) * 2 * H],
                                    start=True, stop=True)
                            # inject gathered hs on DVE: zf = z + hsg
                            zf = ap.tile([128, SUPER * 2 * H], F32, tag="zf")
                            nc.vector.tensor_tensor(
                                out=zf[:, :ns * 2 * H], in0=z[:, :ns * 2 * H],
                                in1=hsg[:, (st0) * 2 * H:(st0 + ns) * 2 * H],
                                op=ALU.add)
                            # msg = sigmoid(zg)*softplus(zf'); zg' = -zg folded
                            a_t = ap.tile([128, SUPER * 2 * H], BF16, tag="a")
                            nc.scalar.activation(a_t[:, :ns * 2 * H],
                                                 zf[:, :ns * 2 * H], AF.Exp)
                            bd = ap.tile([128, SUPER * 2 * H], F32, tag="bd")
                            nc.scalar.activation(bd[:, :ns * 2 * H],
                                                 a_t[:, :ns * 2 * H], AF.Ln,
                                                 bias=1.0)
                            bd3 = bd[:, :ns * 2 * H].rearrange(
                                "p (g c) -> p g c", c=2 * H)
                            e_t = ap.tile([128, SUPER * H], BF16, tag="e")
                            e3 = e_t[:, :ns * H].rearrange(
                                "p (g c) -> p g c", c=H)
                            nc.scalar.activation(e3, bd3[:, :, 0:H], AF.Exp,
                                                 scale=-1.0)
                            msg = ap.tile([128, SUPER * H], BF16, tag="msg")
                            msg3 = msg[:, :ns * H].rearrange(
                                "p (g c) -> p g c", c=H)
                            nc.vector.tensor_tensor(
                                out=msg3, in0=e3, in1=bd3[:, :, H:2 * H],
                                op=ALU.mult)

                            for j in range(ns):
                                g = gbase + j
                                b_j = grp_kb[g][1]
                                if g in run_start:
                                    agg_ps = agp.tile([128, 128], F32,
                                                      space="PSUM",
                                                      tag="aggps")
                                    _, run_b, run_ng = run_start[g]
                                    run_end = g + run_ng - 1
                                nc.tensor.matmul(
                                    agg_ps[:, :BS],
                                    lhsT=msg[:, j * H:(j + 1) * H],
                                    rhs=Ssb[:, j * 128:j * 128 + BS],
                                    start=(g in run_start),
                                    stop=(g == run_end))
                                if g == run_end:
                                    asl = agg_sb[:, run_b * BS:
                                                 (run_b + 1) * BS]
                                    g0r = g - run_ng + 1
                                    if g0r in first_run_g0:
                                        nc.vector.tensor_copy(asl, agg_ps[:, :BS])
                                    else:
                                        nc.vector.tensor_tensor(
                                            out=asl, in0=asl,
                                            in1=agg_ps[:, :BS], op=ALU.add)
                                    if g0r in last_run_g0:
                                        sq = ap.tile([128, BS], F32, tag="sq")
                                        nc.vector.tensor_tensor(
                                            out=sq[:], in0=asl, in1=asl,
                                            op=ALU.mult)
                                        nc.vector.tensor_reduce(
                                            stats1[:, run_b:run_b + 1], asl,
                                            axis=mybir.AxisListType.X,
                                            op=ALU.add)
                                        nc.vector.tensor_reduce(
                                            stats2[:, run_b:run_b + 1], sq[:],
                                            axis=mybir.AxisListType.X,
                                            op=ALU.add)

                    stt = stp.tile([H, 2], F32, tag="stt")
                    nc.vector.tensor_reduce(stt[:, 0:1], stats1[:],
                                            axis=mybir.AxisListType.X,
                                            op=ALU.add)
                    nc.vector.tensor_reduce(stt[:, 1:2], stats2[:],
                                            axis=mybir.AxisListType.X,
                                            op=ALU.add)
                    nc.sync.dma_start(stat_in[:], stt[:])
                nc.gpsimd.collective_compute(
                    "AllReduce", ALU.add, replica_groups=RG,
                    ins=[stat_in[:]], outs=[stat_out[:]])

                # ---- phase S: bn scale/shift ----
                with tc.tile_pool(name="ps2", bufs=1) as sp2:
                    stf = sp2.tile([H, 2], F32, tag="stf")
                    nc.sync.dma_start(stf[:], stat_out[:])
                    mv = sp2.tile([H, 4], F32, tag="mv")
                    nc.vector.tensor_scalar_mul(mv[:, 0:1], stf[:, 0:1], 1.0 / N)
                    nc.vector.tensor_scalar_mul(mv[:, 1:2], stf[:, 1:2], 1.0 / N)
                    nc.vector.tensor_tensor(out=mv[:, 3:4], in0=mv[:, 0:1],
                                            in1=mv[:, 0:1], op=ALU.mult)
                    nc.vector.tensor_tensor(out=mv[:, 2:3], in0=mv[:, 1:2],
                                            in1=mv[:, 3:4], op=ALU.subtract)
                    nc.vector.tensor_scalar_add(mv[:, 2:3], mv[:, 2:3], EPS)
                    lnv = sp2.tile([H, 1], F32, tag="lnv")
                    nc.scalar.activation(lnv[:], mv[:, 2:3], AF.Ln)
                    nc.vector.tensor_scalar_mul(lnv[:], lnv[:], -0.5)
                    rs = sp2.tile([H, 1], F32, tag="rs")
                    nc.scalar.activation(rs[:], lnv[:], AF.Exp)
                    nc.vector.tensor_tensor(out=scsh_sb[:, 0:1], in0=rs[:],
                                            in1=bng_sb[li][:], op=ALU.mult)
                    nc.vector.tensor_tensor(out=mv[:, 3:4], in0=mv[:, 0:1],
                                            in1=scsh_sb[:, 0:1], op=ALU.mult)
                    nc.vector.tensor_tensor(out=scsh_sb[:, 1:2],
                                            in0=bnb_sb[li][:], in1=mv[:, 3:4],
                                            op=ALU.subtract)

                # ---- fused phase U (+ next-layer H, or pooling on last) ----
                last = li == NL - 1
                with (
                    tc.tile_pool(name="pu", bufs=4) as up,
                    tc.tile_pool(name="pup", bufs=2, space="PSUM") as upp,
                    tc.tile_pool(name="pug", bufs=1, space="PSUM") as ugp,
                ):
                    if last:
                        g_pool_psum = ugp.tile([B, H], F32, space="PSUM",
                                               tag="gpool")
                    UB = 4           # 128-node tiles per superblock
                    for t0 in range(0, NB, UB):
                        nt = min(UB, NB - t0)
                        w = nt * 128
                        ht = up.tile([128, UB * 128], BF16, tag="ht")
                        nc.sync.dma_start(
                            ht[:, :w], hT_dram[:, t0 * 128:t0 * 128 + w])
                        t1 = up.tile([128, UB * 128], F32, tag="t1")
                        nc.vector.scalar_tensor_tensor(
                            out=t1[:, :w],
                            in0=agg_sb[:, t0 * 128:t0 * 128 + w],
                            scalar=scsh_sb[:, 0:1],
                            in1=scsh_sb[:, 1:2].to_broadcast([128, w]),
                            op0=ALU.mult, op1=ALU.add)
                        t2 = up.tile([128, UB * 128], F32, tag="t2")
                        nc.vector.tensor_tensor(out=t2[:, :w], in0=t1[:, :w],
                                                in1=ht[:, :w], op=ALU.add)
                        hnew = up.tile([128, UB * 128], BF16, tag="hnew")
                        nc.vector.tensor_scalar_max(hnew[:, :w], t2[:, :w],
                                                    0.0)
                        if not last:
                            nc.sync.dma_start(
                                hT_dram[:, t0 * 128:t0 * 128 + w],
                                hnew[:, :w])
                        for ti in range(nt):
                            t = t0 + ti
                            hv = hnew[:, ti * 128:(ti + 1) * 128]
                            if not last:
                                ps2 = upp.tile([128, 4 * H], F32,
                                               space="PSUM", tag="hsd")
                                nc.tensor.matmul(ps2[:], lhsT=hv,
                                                 rhs=wsd_sb[li + 1][:],
                                                 start=True, stop=True)
                                hs_hd_from_psum(up, ps2, t, li + 1)
                            else:
                                tp = upp.tile([128, 128], BF16, space="PSUM",
                                              tag="tp")
                                nc.tensor.transpose(out=tp[:], in_=hv,
                                                    identity=id_bf[:])
                                hbk = up.tile([128, 128], BF16, tag="hbk")
                                nc.vector.tensor_copy(hbk[:], tp[:])
                                Bt = up.tile([128, B], BF16, tag="Bt")
                                nc.sync.dma_start(Bt[:],
                                                  Bt_d[:, t * B:(t + 1) * B])
                                nc.tensor.matmul(g_pool_psum[:], lhsT=Bt[:],
                                                 rhs=hbk[:], start=(t == 0),
                                                 stop=(t == NB - 1))
                        if not last and t0 < HTILES <= t0 + nt:
                            nc.gpsimd.collective_compute(
                                "AllGather", ALU.bypass, replica_groups=RG,
                                ins=[hsag_in_l[li + 1][0][:]],
                                outs=[hs_full_l[li + 1][0][:]])
                    if last:
                        gsb = up.tile([B, H], F32, tag="gsb")
                        nc.vector.tensor_copy(gsb[:], g_pool_psum[:])
                        nc.sync.dma_start(g_in[:], gsb[:])
                if not last:
                    nc.gpsimd.collective_compute(
                        "AllGather", ALU.bypass, replica_groups=RG,
                        ins=[hsag_in_l[li + 1][1][:]],
                        outs=[hs_full_l[li + 1][1][:]])

            # ============ head ============
            nc.gpsimd.collective_compute(
                "AllReduce", ALU.add, replica_groups=RG,
                ins=[g_in[:]], outs=[g_out[:]])
            with (
                tc.tile_pool(name="hd2", bufs=1) as hp2,
                tc.tile_pool(name="hdp2", bufs=2, space="PSUM") as hpp2,
            ):
                gsum = hp2.tile([B, H], F32, tag="gsum")
                nc.sync.dma_start(gsum[:], g_out[:])
                gmean = hp2.tile([B, H], F32, tag="gmean")
                nc.vector.tensor_scalar_mul(gmean[:], gsum[:], cinv_sb[:, 0:1])
                tps = hpp2.tile([H, B], F32, space="PSUM", tag="tps")
                nc.tensor.transpose(out=tps[:], in_=gmean[:],
                                    identity=id_f32[:B, :B])
                gT = hp2.tile([H, B], F32, tag="gT")
                nc.vector.tensor_copy(gT[:], tps[:])
                x1p = hpp2.tile([H, B], F32, space="PSUM", tag="x1p")
                nc.tensor.matmul(x1p[:], lhsT=fc1w_sb[:], rhs=gT[:],
                                 start=True, stop=True)
                x1 = hp2.tile([H, B], F32, tag="x1")
                nc.vector.scalar_tensor_tensor(
                    out=x1[:], in0=x1p[:], scalar=1.0,
                    in1=fc1b_sb[:].to_broadcast([H, B]),
                    op0=ALU.mult, op1=ALU.add)
                sc2 = hp2.tile([H, 6], F32, tag="sc2")
                nc.vector.tensor_reduce(sc2[:, 0:1], x1[:],
                                        axis=mybir.AxisListType.X, op=ALU.add)
                nc.vector.tensor_scalar_mul(sc2[:, 1:2], sc2[:, 0:1], 1.0 / B)
                xsq = hp2.tile([H, B], F32, tag="xsq")
                nc.vector.tensor_tensor(out=xsq[:], in0=x1[:], in1=x1[:],
                                        op=ALU.mult)
                nc.vector.tensor_reduce(sc2[:, 2:3], xsq[:],
                                        axis=mybir.AxisListType.X, op=ALU.add)
                nc.vector.tensor_scalar_mul(sc2[:, 2:3], sc2[:, 2:3], 1.0 / B)
                nc.vector.tensor_tensor(out=sc2[:, 3:4], in0=sc2[:, 1:2],
                                        in1=sc2[:, 1:2], op=ALU.mult)
                nc.vector.tensor_tensor(out=sc2[:, 3:4], in0=sc2[:, 2:3],
                                        in1=sc2[:, 3:4], op=ALU.subtract)
                nc.vector.tensor_scalar_add(sc2[:, 3:4], sc2[:, 3:4], EPS)
                lnv2 = hp2.tile([H, 1], F32, tag="lnv2")
                nc.scalar.activation(lnv2[:], sc2[:, 3:4], AF.Ln)
                nc.vector.tensor_scalar_mul(lnv2[:], lnv2[:], -0.5)
                rs2 = hp2.tile([H, 1], F32, tag="rs2")
                nc.scalar.activation(rs2[:], lnv2[:], AF.Exp)
                nc.vector.tensor_tensor(out=sc2[:, 4:5], in0=rs2[:],
                                        in1=fcg_sb[:], op=ALU.mult)
                nc.vector.tensor_tensor(out=sc2[:, 3:4], in0=sc2[:, 1:2],
                                        in1=sc2[:, 4:5], op=ALU.mult)
                nc.vector.tensor_tensor(out=sc2[:, 5:6], in0=fcb_sb[:],
                                        in1=sc2[:, 3:4], op=ALU.subtract)
                x2 = hp2.tile([H, B], F32, tag="x2")
                nc.vector.scalar_tensor_tensor(
                    out=x2[:], in0=x1[:], scalar=sc2[:, 4:5],
                    in1=sc2[:, 5:6].to_broadcast([H, B]),
                    op0=ALU.mult, op1=ALU.add)
                x2r = hp2.tile([H, B], F32, tag="x2r")
                nc.vector.tensor_scalar_max(x2r[:], x2[:], 0.0)
                yp = hpp2.tile([1, B], F32, space="PSUM", tag="yp")
                nc.tensor.matmul(yp[:], lhsT=outw_sb[:], rhs=x2r[:],
                                 start=True, stop=True)
                ysb = hp2.tile([1, B], F32, tag="ysb")
                nc.vector.tensor_scalar_add(ysb[:], yp[:], meta["out_b"])
                nc.sync.dma_start(out_d[:].rearrange("b o -> o b"), ysb[:])

    return nc


# --------------------------------------------------------------------------
def run(inputs, cfg=None, debug=False):
    cfg = cfg or DEFAULT_CFG
    meta, in_maps = _preprocess(cfg, inputs)
    nc = _build(cfg, meta, debug=debug)
    nc.finalize()
    res = run_bass_kernel_spmd(nc, in_maps, core_ids=list(range(NCORES)))
    return np.asarray(res.results[0]["out"], np.float32)


def kernel(**inputs):
    return run(inputs, DEFAULT_CFG)
